# revision 43
# baseline (speedup 1.0000x reference)
"""Causal+padding-masked multi-head attention on 8 Trainium2 NeuronCores.

Problem: q[2,16,2048,64], k[2,16,64,2048], v[2,16,2048,64], mask_pad[2,1,1,2048]
-> out[2,16,2048,64] fp32 (softmax((q@k)/8 with pad+causal mask) @ v).

Sharding: batch*head data parallel - 32 (b,h) pairs, 4 per core; cores 0-3
take batch 0, cores 4-7 batch 1.

Pad-masked keys get softmax weight exactly 0 in the reference, so the host
GATHERS only the valid keys per batch (~half of 2048) and the device computes
attention over the packed keys: QK matmuls, exp, and AV matmuls all halve.
The causal boundary in packed-key space is a staircase (packed order
preserves t order). Chunks fully below it need no masking; partial chunks get
one fused DVE op per chunk:
    at[p, s] *= (iota[s] >= t_p - 512j)
with t_p the original key index of packed row p (dummy rows: t_p = 30000).
Columns s < t_min(chunk) - 512j are dead for every batch and are skipped
outright in QK / exp / mask / AV (bf16 matmul has no minimum-width penalty).

Schedule notes (measured on HW): the PE clock gate (HAM) throttles to 1.2GHz
unless the PE stream covers a full free-running ~3.4us activity window with
no >~2us gap; NWARM back-to-back 512-row bf16 warmup matmuls followed
seamlessly (inputs DMAd column-block-first across all three DMA queues) by
the j-descending QK runway keeps it at 2.4 GHz for the whole kernel. ScalarE
(exp) is the near-saturated bottleneck (~41us busy); the DVE absorbs the
staircase masking (prebuilt 0/1 tiles + tensor_mul), the PSUM->SBUF output
copies, and the exp of two interior chunk-pairs per (bh,j>=2) via the
Schraudolph bit trick, bringing both engines to ~40-42us.

Per-core kernel, per (b,h), matmul operands bf16 (1 cyc/row, f32 PSUM):
  scoresT[p,s] = sum_d k_packed[d,p]*qT[d,s]           (K=64)
  at = exp(scoresT * 0.125): one ScalarE act per pair-tile (spans dead
     strips between column-skipped chunks; exp(stale) is never read), or
     int16(scores*A+B) bitcast bf16 on the DVE for the Schraudolph pairs
  staircase mask via DVE tensor_mul with prebuilt masks (exact zeroing)
  oT[d,s] accumulated over packed chunks in PSUM; vx has a ones column
     (0 for dummy rows) so row 64 of oT is the softmax denominator.
  oT copied PSUM->SBUF (DVE) and DMAd out per (bh, j) block.

Host divides by the denominator row and transposes; rows with no valid key
<= s (reference softmaxes a constant row -> uniform -> mean over ALL t of v)
are fixed up on the host from mask_pad + v directly.
"""
import os
import sys

sys.path.insert(0, "/opt/trn_rl_repo")

import numpy as np

B, H, S, D = 2, 16, 2048, 64
NCORES = 8
BH_PER_CORE = (B * H) // NCORES  # 4
NBLK = S // 512     # 4 s-blocks of 512
NWARM = 14  # 512-row bf16 warmup matmuls (~5us): the first real QK group
            # (block j=3, queued behind warmup, inputs DMAd column-block-
            # first) continues the PE stream seamlessly, so a full HAM
            # activity window is covered and the clock gate ramps to 8/8
            # without ever seeing a gap (a >~2us gap re-throttles it)


def _register_ntff_shim():
    """The image's antenv lacks axon_hooks; register the NTFF profile hook so
    BASS_TRACE=1 works. Degrades silently if the axon boot pieces are absent."""
    import types
    if "antenv.axon_hooks" in sys.modules:
        return
    try:
        mod = types.ModuleType("antenv.axon_hooks")
        _hook = [None]
        mod.set_axon_ntff_profile_hook = lambda h: _hook.__setitem__(0, h)
        mod.get_axon_ntff_profile_hook = lambda: _hook[0]
        sys.modules["antenv.axon_hooks"] = mod
        import antenv
        antenv.axon_hooks = mod
        if "/root/.axon_site" not in sys.path:
            sys.path.insert(0, "/root/.axon_site")
        from trn_agent_boot.trn_boot import _ntff_profile_via_ctypes
        mod.set_axon_ntff_profile_hook(
            _ntff_profile_via_ctypes("/opt/axon/libaxon_pjrt.so"))
    except Exception:
        pass


def _plan(mask_bool):
    """Compile-time plan shared by all 8 cores (union over both batches).

    Returns (npkch, C, MS, LO, tvs_index):
      npkch: packed-key chunks of 128
      C[j]: chunks to process for s-block j
      MS[j]: chunk set needing the staircase mask in block j
      LO[j][c]: first live column of chunk c in block j (0 for full chunks)
      tvs_index[(j, c)]: column in the precomputed t-shift tile
    """
    valids = [np.where(mask_bool[b])[0] for b in range(B)]
    nmax = max(1, max(len(v) for v in valids))
    npkch = (nmax + 127) // 128
    npk = npkch * 128
    tv = np.full((B, npk), 1.0e9, np.float64)
    for b in range(B):
        tv[b, :len(valids[b])] = valids[b]
    C, MS, LO = [], [], []
    for j in range(NBLK):
        smin, smax = 512 * j, 512 * j + 511
        cj = 1
        for b in range(B):
            for c in range(npkch):
                if tv[b, 128 * c] <= smax:
                    cj = max(cj, c + 1)
        ms = {c for b in range(B) for c in range(cj)
              if tv[b, 128 * c + 127] > smin}
        lo = [int(max(0, min(512, min(tv[b, 128 * c] for b in range(B)) - smin)))
              for c in range(cj)]
        C.append(cj)
        MS.append(ms)
        LO.append(lo)
    tvs_index = {}
    for j in range(NBLK):
        for c in sorted(MS[j]):
            tvs_index[(j, c)] = len(tvs_index)
    return npkch, C, MS, LO, tvs_index


def build_program(npkch, C, MS, LO, tvs_index):
    import concourse.bacc as bacc
    import concourse.tile as tile
    import concourse.mybir as mybir

    f32 = mybir.dt.float32
    f32r = mybir.dt.float32r
    bf16 = mybir.dt.bfloat16
    AF = mybir.ActivationFunctionType
    ALU = mybir.AluOpType

    NPK = npkch * 128
    NM = max(1, len(tvs_index))
    # Interior full chunk-pairs whose exp runs on the DVE via the Schraudolph
    # bit trick  int16(scores*A + B) bitcast bf16 ~= exp(scores/8)  (~3%
    # weight error confined to <=1/4 of any query row's keys). Balances the
    # saturated ScalarE against the DVE's slack.
    SCHRAUD = {(j, 2) for j in (2, 3)
               if C[j] >= 4 and LO[j][2] == 0 and LO[j][3] == 0}
    SCH_A = float(0.125 * 128 / np.log(2.0))   # folds the 1/sqrt(d) scale
    SCH_B = 16250.75                           # 127*128 + mantissa-interp shift

    nc = bacc.Bacc("TRN2", target_bir_lowering=False, debug=False)

    qt_d = nc.dram_tensor("qt", [BH_PER_CORE, 64, S], bf16, kind="ExternalInput")
    kx_d = nc.dram_tensor("kx", [BH_PER_CORE, 64, NPK], bf16, kind="ExternalInput")
    vx_d = nc.dram_tensor("vx", [BH_PER_CORE, 128, npkch, 65], bf16, kind="ExternalInput")
    tv_d = nc.dram_tensor("tv", [128, npkch], f32, kind="ExternalInput")
    out_d = nc.dram_tensor("out", [BH_PER_CORE, 65, NBLK, 512], f32, kind="ExternalOutput")

    with tile.TileContext(nc) as tc:
        with (
            tc.tile_pool(name="consts", bufs=1) as consts,
            tc.tile_pool(name="qt", bufs=2) as qt_pool,
            tc.tile_pool(name="kx", bufs=2) as kx_pool,
            tc.tile_pool(name="vx", bufs=2) as vx_pool,
            tc.tile_pool(name="at", bufs=6) as at_pool,
            tc.tile_pool(name="osb", bufs=4) as osb_pool,
            tc.tile_pool(name="ps_s", bufs=3, space="PSUM") as ps_s,
            tc.tile_pool(name="ps_o", bufs=2, space="PSUM") as ps_o,
        ):
            # warm-up constant memset on the (otherwise idle) DVE so the PE
            # warm-up matmuls can start as early as possible, while the input
            # DMAs are in flight on the sync/scalar/gpsimd queues.
            warm_sb = consts.tile([128, 512], bf16)
            nc.vector.memset(warm_sb[:], 0.01)

            # bh0 inputs split across the three DMA-capable queues, qt in
            # column blocks high-j first (blocks are processed j-descending),
            # so the first QK group's data lands right as warmup drains.
            qt0 = qt_pool.tile([64, S], bf16)
            kx0 = kx_pool.tile([64, NPK], bf16)
            vx0 = vx_pool.tile([128, npkch, 65], bf16)
            nc.sync.dma_start(qt0[:, 1536:2048], qt_d[0, :, 1536:2048])
            nc.scalar.dma_start(kx0[:], kx_d[0])
            nc.gpsimd.dma_start(vx0[:], vx_d[0])
            nc.sync.dma_start(qt0[:, 512:1024], qt_d[0, :, 512:1024])
            nc.scalar.dma_start(qt0[:, 1024:1536], qt_d[0, :, 1024:1536])
            nc.scalar.dma_start(qt0[:, 0:512], qt_d[0, :, 0:512])
            tv_sb = consts.tile([128, npkch], f32)
            nc.gpsimd.dma_start(tv_sb[:], tv_d[:])

            iota_sb = consts.tile([128, 512], f32)
            nc.gpsimd.iota(iota_sb[:], pattern=[[1, 512]], base=0,
                           channel_multiplier=0,
                           allow_small_or_imprecise_dtypes=True)

            # per masked (j, c): prebuilt 0/1 staircase mask
            #   mask[p, s] = (iota[s] + 512j >= t_p)
            # applied later with a plain (all-SBUF, 2-byte) tensor_mul
            msk = consts.tile([128, NM, 512], bf16)
            # build in descending-j order to match block processing order, so
            # the first block's masks are ready before its first tensor_mul
            for (j, c), idx in sorted(tvs_index.items(), key=lambda kv: -kv[0][0]):
                nc.vector.tensor_scalar(
                    msk[:, idx, :], iota_sb[:], 512.0 * j,
                    tv_sb[:, c:c + 1], op0=ALU.add, op1=ALU.is_ge)

            # Contiguous 512-row bf16 dummy matmuls: long enough to cover a
            # full (free-running) HAM activity window, so the PE clock gate
            # ramps to 8/8 during warmup and real matmuls run at 2.4 GHz.
            warm_ps = ps_s.tile([128, 1024], f32, tag="sc")
            for w in range(NWARM):
                nc.tensor.matmul(
                    warm_ps[:, 0:512], warm_sb[:, 0:128], warm_sb[:],
                    start=(w == 0), stop=(w == NWARM - 1), skip_group_check=True)

            for l in range(BH_PER_CORE):
                if l == 0:
                    qt_sb, kx_sb, vx_sb = qt0, kx0, vx0
                else:
                    qt_sb = qt_pool.tile([64, S], bf16)
                    kx_sb = kx_pool.tile([64, NPK], bf16)
                    vx_sb = vx_pool.tile([128, npkch, 65], bf16)
                    nc.sync.dma_start(qt_sb[:], qt_d[l])
                    nc.sync.dma_start(kx_sb[:], kx_d[l])
                    nc.gpsimd.dma_start(vx_sb[:], vx_d[l])

                jorder = range(NBLK - 1, -1, -1)  # big blocks first: deep QK runway
                # keeps the PE stream dense (HAM stays 8/8); small j=0 last
                # shortens the per-bh tail
                for j in jorder:
                    cj = C[j]
                    oT_ps = ps_o.tile([65, 512], f32)
                    for c0 in range(0, cj, 2):
                        w = min(2, cj - c0)
                        lows = [LO[j][c0 + ci] for ci in range(w)]
                        sc_ps = ps_s.tile([128, 1024], f32, tag="sc")
                        for ci in range(w):
                            lo = lows[ci]
                            nc.tensor.matmul(
                                sc_ps[:, 512 * ci + lo:512 * (ci + 1)],
                                kx_sb[:, 128 * (c0 + ci):128 * (c0 + ci + 1)],
                                qt_sb[:, 512 * j + lo:512 * (j + 1)],
                                start=True, stop=True)
                        at = at_pool.tile([128, 1024], bf16)
                        # the very first block (l=0, top j) must not depend on
                        # the DVE, which is still building masks then — a DVE
                        # stall there gaps the PE stream and re-throttles HAM
                        if ((j, c0) in SCHRAUD and w == 2
                                and not (l == 0 and j == NBLK - 1)):
                            nc.vector.tensor_scalar(
                                at[:].bitcast(mybir.dt.int16), sc_ps[:],
                                SCH_A, SCH_B, op0=ALU.mult, op1=ALU.add)
                        else:
                            # ONE act per pair-tile, spanning from the first
                            # live column through the end: the dead strip
                            # between a full chunk and a column-skipped
                            # partial gets exp(stale PSUM), which the AV
                            # matmuls never read — one instruction's fixed
                            # cost beats the extra columns
                            nc.scalar.activation(
                                at[:, lows[0]:512 * w], sc_ps[:, lows[0]:512 * w],
                                AF.Exp, bias=0.0, scale=0.125)
                        for ci in range(w):
                            c = c0 + ci
                            lo = lows[ci]
                            if c in MS[j]:
                                sl = at[:, 512 * ci + lo:512 * (ci + 1)]
                                # Pool takes the high-j masks (it is nearly
                                # idle; its ~3x slower mul hides behind the
                                # deep at-buffering), DVE the rest
                                eng = nc.gpsimd if j >= 2 else nc.vector
                                eng.tensor_mul(
                                    sl, sl,
                                    msk[:, tvs_index[(j, c)], lo:512])
                            nc.tensor.matmul(
                                oT_ps[:, lo:512], vx_sb[:, c, :],
                                at[:, 512 * ci + lo:512 * (ci + 1)],
                                start=(c == 0), stop=(c == cj - 1))
                    oT_sb = osb_pool.tile([65, 512], f32)
                    nc.vector.tensor_copy(oT_sb[:], oT_ps[:])
                    if l == BH_PER_CORE - 1 and j == 0:
                        # final block: split the out DMA across two queues so
                        # the tail transfer halves
                        nc.sync.dma_start(out_d[l, :, j, 0:256], oT_sb[:, 0:256])
                        nc.gpsimd.dma_start(out_d[l, :, j, 256:512], oT_sb[:, 256:512])
                    else:
                        nc.sync.dma_start(out_d[l, :, j, :], oT_sb[:])

    nc.compile()
    return nc


_PROGRAM = None
_PROGRAM_KEY = None
LAST_RESULTS = None


def kernel(q, k, v, mask_pad):
    global _PROGRAM, _PROGRAM_KEY, LAST_RESULTS
    from ml_dtypes import bfloat16 as np_bf16
    q = np.ascontiguousarray(np.asarray(q, dtype=np.float32))
    k = np.ascontiguousarray(np.asarray(k, dtype=np.float32))
    v = np.ascontiguousarray(np.asarray(v, dtype=np.float32))
    mask_pad = np.asarray(mask_pad)

    if os.environ.get("BASS_TRACE"):
        _register_ntff_shim()

    mask_bool = mask_pad[:, 0, 0, :] != 0  # [B, S]
    npkch, C, MS, LO, tvs_index = _plan(mask_bool)
    NPK = npkch * 128
    valids = [np.where(mask_bool[b])[0] for b in range(B)]

    # packed t values (dummy rows: 1e9 so the staircase mask drops them)
    tvv = np.full((B, NPK), 30000.0, np.float32)  # int16-safe dummy sentinel
    for b in range(B):
        tvv[b, :len(valids[b])] = valids[b].astype(np.float32)

    in_maps = []
    for core in range(NCORES):
        b0 = (core * BH_PER_CORE) // H
        val = valids[b0]
        n = len(val)
        qt = np.empty((BH_PER_CORE, 64, S), np_bf16)
        kx = np.zeros((BH_PER_CORE, 64, NPK), np_bf16)
        vx = np.zeros((BH_PER_CORE, 128, npkch, 65), np_bf16)
        for l in range(BH_PER_CORE):
            bh = core * BH_PER_CORE + l
            bb, h = bh // H, bh % H
            qt[l] = q[bb, h].T.astype(np_bf16)
            kx[l, :, :n] = k[bb, h][:, val].astype(np_bf16)
            vg = np.zeros((NPK, 65), np.float32)
            vg[:n, :D] = v[bb, h][val]
            vg[:n, D] = 1.0
            vx[l] = vg.reshape(npkch, 128, 65).transpose(1, 0, 2).astype(np_bf16)
        tvt = tvv[b0].reshape(npkch, 128).T.copy()  # [128, npkch]
        in_maps.append({"qt": qt, "kx": kx, "vx": vx, "tv": tvt})

    key = (npkch, tuple(C), tuple(map(tuple, (sorted(m) for m in MS))),
           tuple(map(tuple, LO)))
    if _PROGRAM is None or _PROGRAM_KEY != key:
        _PROGRAM = build_program(npkch, C, MS, LO, tvs_index)
        _PROGRAM_KEY = key

    from concourse.bass_utils import run_bass_kernel_spmd
    res = run_bass_kernel_spmd(_PROGRAM, in_maps, core_ids=list(range(NCORES)))
    LAST_RESULTS = res
    if res.exec_time_ns is not None:
        print(f"HW exec time: {res.exec_time_ns} ns")
        if res.profile_json:
            print(f"profile_json: {res.profile_json}")

    out = np.empty((B, H, S, D), np.float32)
    bad_rows = [np.where(np.cumsum(mask_bool[b]) == 0)[0] for b in range(B)]
    for core in range(NCORES):
        o = res.results[core]["out"]  # [BH_PER_CORE, 65, NBLK, 512]
        for l in range(BH_PER_CORE):
            bh = core * BH_PER_CORE + l
            bb, h = bh // H, bh % H
            oT = np.asarray(o[l], np.float32).reshape(65, S)
            with np.errstate(divide="ignore", invalid="ignore"):
                res_bh = (oT[:D] / oT[D:D + 1]).T
            bad = bad_rows[bb]
            if len(bad):
                res_bh[bad] = v[bb, h].mean(axis=0)
            out[bb, h] = res_bh
    return out
